# revision 3
# baseline (speedup 1.0000x reference)
"""Trainium2 Bass kernel for dual-input complement-softmax attention (fp8).

Same algebra as the f32r baseline, but with fp8e4m3 operands everywhere on
the PE so the big matmuls (projections, VW, PV) run in DoubleRow perf mode
(256-deep contraction at 0.5 cycles/row = 4x f32r/bf16 throughput).

Scaling scheme (host folds into weights; relu commutes with positive scale):
  q' = 32*q_true (s_attn folded too), k' = 8*k, v' = 8*v, Wp' = 4*Wp
  dots_psum = 256*dots_true  -> exp via activation(Exp, scale=1/256, bias=-1.5)
  P = exp(dots - 1.5) in e4m3 (real dots lie in [0.1, 6.9]; P in [0.25, 230])
  VW = v'@Wp'^T = 32*VW_true (e4m3, |VW|max ~ 440 < 448)
  out = relu((h - F/r)/32), descale folded into the final DVE ops.
h is computed from f32 sums of the quantized v and an f32 copy of the
*dequantized* fp8 Wp so the dominant h term carries no fp8 weight error.

Sharding: 8 cores = 4 batches x 2 query-row halves, no collectives.
Output is written n-major [NQ, 256]; host transposes.
"""

import numpy as np
import ml_dtypes

B, C, HH, WW = 4, 256, 64, 64
N = HH * WW        # 4096 keys per batch
NQ = N // 2        # 2048 query rows per core
INTER = 128
OUT = 256
NCORES = 8

AQ, AK, AV, AP_ = 32.0, 8.0, 8.0, 4.0
EXP_BIAS = -1.5
DOTS_DESCALE = 1.0 / (AQ * AK)
OUT_DESCALE = 1.0 / (AV * AP_)
# Schraudolph fast-exp for the offloaded tiles (DVE affine -> int32, Pool
# bitcast-convert to fp8): int32(dots_psum*EXPA + EXPB) ~= exp(dots - 1.5)
_LOG2E_SCALE = 2.0 ** 23 / float(np.log(2.0))
EXPA = _LOG2E_SCALE / (AQ * AK)
EXPB = 127.0 * 2.0 ** 23 - 366000.0 + EXP_BIAS * _LOG2E_SCALE

_NC_CACHE = {}


def _build_nc():
    import concourse.bacc as bacc
    import concourse.mybir as mybir
    import concourse.tile as tile

    f32 = mybir.dt.float32
    f32r = mybir.dt.float32r
    fp8 = mybir.dt.float8e4
    i32 = mybir.dt.int32
    A = mybir.AluOpType
    AF = mybir.ActivationFunctionType
    X = mybir.AxisListType.X
    XY = mybir.AxisListType.XY
    DR = mybir.MatmulPerfMode.DoubleRow

    nc = bacc.Bacc(None, target_bir_lowering=False)

    x12 = nc.dram_tensor("x12", [2 * C, N], fp8, kind="ExternalInput")
    # DoubleRow weight pairs: [cin128, proj(q,k,v), pair, cout128]
    wqkv = nc.dram_tensor("wqkv", [128, 3, 2, 128], fp8, kind="ExternalInput")
    wp8 = nc.dram_tensor("wp8", [128, 2, OUT], fp8, kind="ExternalInput")
    wph = nc.dram_tensor("wph", [128, 2, OUT], f32, kind="ExternalInput")
    bqkv = nc.dram_tensor("bqkv", [3 * INTER, 1], f32, kind="ExternalInput")
    bp_row = nc.dram_tensor("bp_row", [1, OUT], f32, kind="ExternalInput")

    out = nc.dram_tensor("out", [NQ, OUT], f32, kind="ExternalOutput")

    MCHUNKS = N // 128           # 32 key chunks
    SB = 512                     # query superblock
    NSB = NQ // SB               # 4

    with tile.TileContext(nc) as tc:
        with (
            tc.tile_pool(name="persist", bufs=1) as persist,
            tc.tile_pool(name="ep", bufs=52) as ep,
            tc.tile_pool(name="yp", bufs=3) as yp,
            tc.tile_pool(name="zp", bufs=8) as zp,
            tc.tile_pool(name="ps", bufs=3, space="PSUM") as ps,
            tc.tile_pool(name="sm", bufs=2, space="PSUM") as sm,
        ):
            # ---- persistent tiles ----
            x_sb = persist.tile([128, 4, N], fp8)   # [c, (x1|x2 pairs), m]
            wqkv_sb = persist.tile([128, 3, 2, 128], fp8)
            wp_sb = persist.tile([128, 2, OUT], fp8)
            wph_sb = persist.tile([128, 2, OUT], f32r)
            bqkv_sb = persist.tile([128, 3], f32)
            bp_sb = persist.tile([1, OUT], f32)
            k2_sb = persist.tile([128, N], fp8)              # [c, m]
            q_sb = persist.tile([128, NQ], fp8)              # [c, n]
            v_sb = persist.tile([128, MCHUNKS, 2, 128], fp8)  # [c, j, v2|v1, m]
            vw_sb = persist.tile([128, MCHUNKS, 258], fp8)   # [m%128, j, o|1,1]
            h_row = persist.tile([1, OUT], f32)
            h_bc = persist.tile([128, OUT], f32)
            ebias = persist.tile([128, 1], f32)
            nc.vector.memset(ebias[:], EXP_BIAS)
            nc.vector.memset(vw_sb[:, :, 256:258], 1.0)

            def xload(s2):
                msl = slice(s2 * 1024, (s2 + 1) * 1024)
                nc.sync.dma_start(
                    x_sb[:, :, msl],
                    x12[:, msl].rearrange("(a p) n -> p a n", p=128))

            # ---- DMA front: slab-granular, critical-path first ----
            nc.sync.dma_start(wqkv_sb[:], wqkv[:])
            nc.sync.dma_start(bqkv_sb[:], bqkv[:].rearrange("(a p) o -> p (a o)", p=128))
            xload(0)
            xload(1)
            xload(2)
            xload(3)
            nc.sync.dma_start(wp_sb[:], wp8[:])
            nc.sync.dma_start(wph_sb[:], wph[:].bitcast(f32r))
            nc.sync.dma_start(bp_sb[:], bp_row[:])

            def proj(a, xoff, s2):
                """psum[:, 0:1024] = W[a] pair-contracted with x slab s2;
                xoff 0 = x1, 2 = x2."""
                psum = ps.tile([128, 1024], f32, tag="ps", name="pp")
                for hf in (0, 1):
                    o = s2 * 1024 + hf * 512
                    nc.tensor.matmul(psum[:, hf * 512:(hf + 1) * 512],
                                     wqkv_sb[:, a],
                                     x_sb[:, xoff:xoff + 2, o:o + 512],
                                     start=True, stop=True, perf_mode=DR)
                return psum

            def vw_mms(s2):
                """8 VW chunks for the 1024-wide m-slab s2 (DoubleRow pairs),
                two chunks per psum tile to batch the DVE copies."""
                for mc in range(0, 8, 2):
                    j = s2 * 8 + mc
                    vwps = sm.tile([128, 512], f32, tag="small", name="vwps")
                    for d in (0, 1):
                        nc.tensor.matmul(vwps[:, d * 256:(d + 1) * 256],
                                         v_sb[:, j + d], wp_sb[:],
                                         start=True, stop=True, perf_mode=DR)
                    nc.vector.tensor_copy(
                        vw_sb[:, j:j + 2, 0:256],
                        vwps[:].rearrange("p (d o) -> p d o", d=2))

            def v1_block(s2, with_q):
                if with_q:
                    qps = proj(0, 0, s2)
                    nc.vector.tensor_scalar(q_sb[:, s2 * 1024:(s2 + 1) * 1024],
                                            qps[:], bqkv_sb[:, 0:1], 0.0,
                                            A.add, A.max)
                v1ps = proj(2, 0, s2)
                nc.vector.tensor_scalar(v_sb[:, 8 * s2:8 * s2 + 8, 1, :],
                                        v1ps[:], bqkv_sb[:, 2:3], 0.0,
                                        A.add, A.max)
                vw_mms(s2)

            exp_map = {sb: [None] * (MCHUNKS // 2) for sb in range(NSB)}

            def dots_slab(sbs, s2):
                """dps+exp for m-chunks of slab s2, for each query superblock.
                Every 4th tile (s2>0) bypasses the ACT engine: DVE applies the
                Schraudolph affine into int32, Pool bitcasts/converts to fp8."""
                for sb in sbs:
                    nsl = slice(sb * SB, (sb + 1) * SB)
                    for jl in range(4):
                        jj = s2 * 4 + jl
                        dps = ps.tile([128, 1024], f32, tag="ps", name="dps")
                        for u in (0, 1):
                            j = jj * 2 + u
                            nc.tensor.matmul(dps[:, u * 512:(u + 1) * 512],
                                             k2_sb[:, j * 128:(j + 1) * 128],
                                             q_sb[:, nsl], start=True, stop=True)
                        et = ep.tile([128, 1024], fp8, tag="exp", name="et")
                        if jl == 3 and sb >= 2:
                            yt = yp.tile([128, 1024], i32, tag="y", name="yt")
                            nc.vector.tensor_scalar(yt[:], dps[:], EXPA, EXPB,
                                                    A.mult, A.add)
                            nc.gpsimd.tensor_copy(et[:], yt[:].bitcast(f32))
                        else:
                            nc.scalar.activation(et[:], dps[:], AF.Exp,
                                                 bias=ebias[:],
                                                 scale=DOTS_DESCALE)
                        exp_map[sb][jj] = et

            def pv_out(sb):
                tiles = exp_map.pop(sb)
                for t in range(SB // 128):
                    nt = sb * 4 + t
                    fps = sm.tile([128, 258], f32, tag="small", name="fps")
                    for jj in range(MCHUNKS // 2):
                        etp = tiles[jj][:].rearrange("p (u n) -> p u n", u=2)
                        nc.tensor.matmul(fps[:], etp[:, :, t * 128:(t + 1) * 128],
                                         vw_sb[:, 2 * jj:2 * jj + 2, 0:258],
                                         start=(jj == 0), stop=(jj == 15),
                                         perf_mode=DR)
                    rn = zp.tile([128, 1], f32, tag="rn")
                    nc.vector.reciprocal(rn[:], fps[:, 256:257])
                    t2 = zp.tile([128, OUT], f32, tag="t2")
                    nc.vector.scalar_tensor_tensor(t2[:], fps[:, 0:256], rn[:],
                                                   h_bc[:], A.mult, A.subtract)
                    z = zp.tile([128, OUT], f32, tag="z")
                    nc.gpsimd.tensor_scalar(z[:], t2[:], -OUT_DESCALE, 0.0,
                                            A.mult, A.max)
                    nc.sync.dma_start(out[nt * 128:(nt + 1) * 128, :], z[:])

            # ---- phase 0: projections + VW + dots/exp for sb0 and sb1 ----
            qps0 = proj(0, 0, 0)
            nc.vector.tensor_scalar(q_sb[:, 0:1024], qps0[:], bqkv_sb[:, 0:1],
                                    0.0, A.add, A.max)
            for s2 in range(4):
                sl = slice(s2 * 1024, (s2 + 1) * 1024)
                kps = proj(1, 2, s2)
                nc.vector.tensor_scalar(k2_sb[:, sl], kps[:], bqkv_sb[:, 1:2],
                                        0.0, A.add, A.max)
                vps = proj(2, 2, s2)
                nc.vector.tensor_scalar(v_sb[:, 8 * s2:8 * s2 + 8, 0, :],
                                        vps[:], bqkv_sb[:, 2:3], 0.0,
                                        A.add, A.max)
                v1_block(s2, with_q=(s2 == 1))
                dots_slab((0, 1), s2)

            # ---- sumv totals, h ----
            sv2f = zp.tile([128, 1], f32, tag="svf")
            sv1f = zp.tile([128, 1], f32, tag="svf")
            nc.vector.tensor_reduce(sv2f[:], v_sb[:, :, 0, :], XY, A.add)
            nc.vector.tensor_reduce(sv1f[:], v_sb[:, :, 1, :], XY, A.add)
            hps = sm.tile([128, 258], f32, tag="small", name="hps")
            nc.tensor.matmul(hps[0:1, 0:256], sv2f[:], wph_sb[:, 0].bitcast(f32),
                             start=True, stop=False)
            nc.tensor.matmul(hps[0:1, 0:256], sv1f[:], wph_sb[:, 1].bitcast(f32),
                             start=False, stop=True)
            nc.vector.tensor_tensor(h_row[:], hps[0:1, 0:256], bp_sb[:], A.add)
            nc.gpsimd.partition_broadcast(h_bc[:], h_row[:])

            # ---- phase 1 steady state: PV(k) overlaps dots(k+2) exps ----
            for s2 in range(4):
                dots_slab((2,), s2)
            pv_out(0)
            for s2 in range(4):
                dots_slab((3,), s2)
            pv_out(1)
            pv_out(2)
            pv_out(3)

    nc.compile()
    return nc


def _host_prep(inputs):
    E4 = ml_dtypes.float8_e4m3fn
    s_attn = np.float32(INTER ** -0.5)
    x1 = np.asarray(inputs["x1"], np.float32).reshape(B, C, N)
    x2 = np.asarray(inputs["x2"], np.float32).reshape(B, C, N)
    x1_8 = x1.astype(E4)
    x2_8 = x2.astype(E4)

    def eff(Wn, bn, sn, tn, extra=np.float32(1.0)):
        Wm = np.asarray(inputs[Wn], np.float32)
        bb = np.asarray(inputs[bn], np.float32)
        ss = np.asarray(inputs[sn], np.float32)
        tt = np.asarray(inputs[tn], np.float32)
        W_eff = (ss[:, None] * Wm) * extra
        b_eff = (ss * bb + tt) * extra
        return np.ascontiguousarray(W_eff.T), b_eff   # W_eff.T: [cin, cout]

    wqT, bqe = eff("Wq", "bq", "sq", "tq", s_attn * np.float32(AQ))
    wkT, bke = eff("Wk", "bk", "sk", "tk", np.float32(AK))
    wvT, bve = eff("Wv", "bv", "sv", "tv", np.float32(AV))
    wpT, bpe = eff("Wp", "bp", "sp", "tp", np.float32(AP_))

    # DoubleRow pair layout [cin128, proj, pair, cout]
    wqkv8 = np.stack([w.reshape(2, 128, 128).transpose(1, 0, 2)
                      for w in (wqT, wkT, wvT)], axis=1).astype(E4)
    wpT_pair = wpT.reshape(2, 128, OUT).transpose(1, 0, 2)
    wp8 = wpT_pair.astype(E4)
    wph = np.ascontiguousarray(wpT_pair, np.float32)   # true f32 Wp for h

    common = dict(
        wqkv=np.ascontiguousarray(wqkv8),
        wp8=np.ascontiguousarray(wp8),
        wph=np.ascontiguousarray(wph),
        bqkv=np.concatenate([bqe, bke, bve]).reshape(3 * INTER, 1),
        bp_row=(bpe * np.float32(AV)).reshape(1, OUT),
    )
    in_maps = []
    for c in range(NCORES):
        b, half = c // 2, c % 2
        # m-axis permutation: own query half first (identical for x1 and x2,
        # so all sum-over-m quantities are unchanged)
        perm = (np.r_[NQ:N, 0:NQ] if half else np.r_[0:N]).astype(np.intp)
        in_maps.append(dict(
            x12=np.ascontiguousarray(
                np.concatenate([x1_8[b][:, perm], x2_8[b][:, perm]], axis=0)),
            **common,
        ))
    return in_maps


def kernel(**inputs):
    from concourse.bass_utils import run_bass_kernel_spmd

    if "nc" not in _NC_CACHE:
        _NC_CACHE["nc"] = _build_nc()
    nc = _NC_CACHE["nc"]

    in_maps = _host_prep(inputs)
    res = run_bass_kernel_spmd(nc, in_maps, core_ids=list(range(NCORES)))

    full = np.empty((B, OUT, N), dtype=np.float32)
    for c in range(NCORES):
        b, half = c // 2, c % 2
        full[b][:, half * NQ:(half + 1) * NQ] = res.results[c]["out"].T
    return full.reshape(B, OUT, HH, WW)


if __name__ == "__main__":
    rng = np.random.default_rng(0)
    fake = {}
    fake["x1"] = rng.standard_normal((B, C, HH, WW), dtype=np.float32)
    fake["x2"] = rng.standard_normal((B, C, HH, WW), dtype=np.float32)
    for k, oc in (("q", INTER), ("k", INTER), ("v", INTER), ("p", OUT)):
        ic = C if k != "p" else 2 * INTER
        fake["W" + k] = rng.standard_normal((oc, ic), dtype=np.float32) * ic ** -0.5
        fake["b" + k] = np.zeros(oc, np.float32)
        fake["s" + k] = rng.uniform(0.5, 1.5, oc).astype(np.float32)
        fake["t" + k] = rng.standard_normal(oc, dtype=np.float32) * 0.1
    o = kernel(**fake)
    print("kernel ran, out shape", o.shape)


# revision 4
# speedup vs baseline: 1.0901x; 1.0901x over previous
"""Trainium2 Bass kernel for dual-input complement-softmax attention (fp8).

Same algebra as the f32r baseline, but with fp8e4m3 operands everywhere on
the PE so the big matmuls (projections, VW, PV) run in DoubleRow perf mode
(256-deep contraction at 0.5 cycles/row = 4x f32r/bf16 throughput).

Scaling scheme (host folds into weights; relu commutes with positive scale):
  q' = 32*q_true (s_attn folded too), k' = 8*k, v' = 8*v, Wp' = 4*Wp
  dots_psum = 256*dots_true  -> exp via activation(Exp, scale=1/256, bias=-1.5)
  P = exp(dots - 1.5) in e4m3 (real dots lie in [0.1, 6.9]; P in [0.25, 230])
  VW = v'@Wp'^T = 32*VW_true (e4m3, |VW|max ~ 440 < 448)
  out = relu((h - F/r)/32), descale folded into the final DVE ops.
h is computed from f32 sums of the quantized v and an f32 copy of the
*dequantized* fp8 Wp so the dominant h term carries no fp8 weight error.

Sharding: 8 cores = 4 batches x 2 query-row halves, no collectives.
Output is written n-major [NQ, 256]; host transposes.
"""

import numpy as np
import ml_dtypes

B, C, HH, WW = 4, 256, 64, 64
N = HH * WW        # 4096 keys per batch
NQ = N // 2        # 2048 query rows per core
INTER = 128
OUT = 256
NCORES = 8

AQ, AK, AV, AP_ = 32.0, 8.0, 8.0, 4.0
EXP_BIAS = -1.5
DOTS_DESCALE = 1.0 / (AQ * AK)
OUT_DESCALE = 1.0 / (AV * AP_)
# Schraudolph fast-exp for the offloaded tiles (DVE affine -> int32, Pool
# bitcast-convert to fp8): int32(dots_psum*EXPA + EXPB) ~= exp(dots - 1.5)
_LOG2E_SCALE = 2.0 ** 23 / float(np.log(2.0))
EXPA = _LOG2E_SCALE / (AQ * AK)
EXPB = 127.0 * 2.0 ** 23 - 366000.0 + EXP_BIAS * _LOG2E_SCALE

_NC_CACHE = {}


def _build_nc():
    import concourse.bacc as bacc
    import concourse.mybir as mybir
    import concourse.tile as tile

    f32 = mybir.dt.float32
    f32r = mybir.dt.float32r
    fp8 = mybir.dt.float8e4
    i32 = mybir.dt.int32
    A = mybir.AluOpType
    AF = mybir.ActivationFunctionType
    X = mybir.AxisListType.X
    XY = mybir.AxisListType.XY
    DR = mybir.MatmulPerfMode.DoubleRow

    nc = bacc.Bacc(None, target_bir_lowering=False)

    x12 = nc.dram_tensor("x12", [2 * C, N], fp8, kind="ExternalInput")
    # DoubleRow weight pairs: [cin128, proj(q,k,v), pair, cout128]
    wqkv = nc.dram_tensor("wqkv", [128, 3, 2, 128], fp8, kind="ExternalInput")
    wp8 = nc.dram_tensor("wp8", [128, 2, OUT], fp8, kind="ExternalInput")
    wph = nc.dram_tensor("wph", [128, 2, OUT], f32, kind="ExternalInput")
    bqkv = nc.dram_tensor("bqkv", [3 * INTER, 1], f32, kind="ExternalInput")
    bp_row = nc.dram_tensor("bp_row", [1, OUT], f32, kind="ExternalInput")

    out = nc.dram_tensor("out", [NQ, OUT], f32, kind="ExternalOutput")

    MCHUNKS = N // 128           # 32 key chunks
    SB = 512                     # query superblock
    NSB = NQ // SB               # 4

    with tile.TileContext(nc) as tc:
        with (
            tc.tile_pool(name="persist", bufs=1) as persist,
            tc.tile_pool(name="ep", bufs=60) as ep,
            tc.tile_pool(name="yp", bufs=3) as yp,
            tc.tile_pool(name="zp", bufs=8) as zp,
            tc.tile_pool(name="ps", bufs=3, space="PSUM") as ps,
            tc.tile_pool(name="sm", bufs=2, space="PSUM") as sm,
        ):
            # ---- persistent tiles ----
            x_sb = persist.tile([128, 4, N], fp8)   # [c, (x1|x2 pairs), m]
            wqkv_sb = persist.tile([128, 3, 2, 128], fp8)
            wp_sb = persist.tile([128, 2, OUT], fp8)
            wph_sb = persist.tile([128, 2, OUT], f32r)
            bqkv_sb = persist.tile([128, 3], f32)
            bp_sb = persist.tile([1, OUT], f32)
            k2_sb = persist.tile([128, N], fp8)              # [c, m]
            q_sb = persist.tile([128, NQ], fp8)              # [c, n]
            v_sb = persist.tile([128, MCHUNKS, 2, 128], fp8)  # [c, j, v2|v1, m]
            vw_sb = persist.tile([128, MCHUNKS, 258], fp8)   # [m%128, j, o|1,1]
            svp = persist.tile([128, 2, 4], f32)     # per-slab v-sum partials
            h_row = persist.tile([1, OUT], f32)
            h_bc = persist.tile([128, OUT], f32)
            ebias = persist.tile([128, 1], f32)
            scr = persist.tile([128, 1], f32)
            nc.vector.memset(ebias[:], EXP_BIAS)
            nc.vector.memset(scr[:], 0.0)
            nc.vector.memset(vw_sb[:, :, 256:258], 1.0)
            # dummy activation: absorbs the ACT table load at t~0
            nc.scalar.activation(scr[:], scr[:], AF.Relu, bias=ebias[:])

            def xload(s2):
                msl = slice(s2 * 1024, (s2 + 1) * 1024)
                nc.sync.dma_start(
                    x_sb[:, :, msl],
                    x12[:, msl].rearrange("(a p) n -> p a n", p=128))

            # ---- DMA front: slab-granular, critical-path first ----
            nc.sync.dma_start(wqkv_sb[:], wqkv[:])
            nc.sync.dma_start(bqkv_sb[:], bqkv[:].rearrange("(a p) o -> p (a o)", p=128))
            for hq in (0, 1):
                qsl = slice(hq * 512, (hq + 1) * 512)
                nc.sync.dma_start(
                    x_sb[:, :, qsl],
                    x12[:, qsl].rearrange("(a p) n -> p a n", p=128))
            xload(1)
            xload(2)
            xload(3)
            nc.sync.dma_start(wp_sb[:], wp8[:])
            nc.sync.dma_start(wph_sb[:], wph[:].bitcast(f32r))
            nc.sync.dma_start(bp_sb[:], bp_row[:])

            def proj_relu(a, xoff, s2, dst_half, bias, act=False):
                """Project slab s2 of x (xoff 0 = x1, 2 = x2) with weight set
                a, in two half-width psum tiles, relu+bias into dst_half(hf).
                act=True runs the relu on the ACT engine (ramp window)."""
                for hf in (0, 1):
                    o = s2 * 1024 + hf * 512
                    psum = sm.tile([128, 512], f32, tag="small", name="pp")
                    nc.tensor.matmul(psum[:], wqkv_sb[:, a],
                                     x_sb[:, xoff:xoff + 2, o:o + 512],
                                     start=True, stop=True, perf_mode=DR)
                    if act:
                        nc.scalar.activation(dst_half(hf), psum[:], AF.Relu,
                                             bias=bias)
                    else:
                        nc.vector.tensor_scalar(dst_half(hf), psum[:], bias,
                                                0.0, A.add, A.max)

            def vw_mms(s2):
                """8 VW chunks for the 1024-wide m-slab s2 (DoubleRow pairs),
                two chunks per psum tile to batch the DVE copies."""
                for mc in range(0, 8, 2):
                    j = s2 * 8 + mc
                    vwps = sm.tile([128, 512], f32, tag="small", name="vwps")
                    for d in (0, 1):
                        nc.tensor.matmul(vwps[:, d * 256:(d + 1) * 256],
                                         v_sb[:, j + d], wp_sb[:],
                                         start=True, stop=True, perf_mode=DR)
                    nc.vector.tensor_copy(
                        vw_sb[:, j:j + 2, 0:256],
                        vwps[:].rearrange("p (d o) -> p d o", d=2))

            def v1_block(s2, with_q):
                if with_q:
                    proj_relu(0, 0, s2,
                              lambda hf: q_sb[:, s2 * 1024 + hf * 512:
                                              s2 * 1024 + (hf + 1) * 512],
                              bqkv_sb[:, 0:1])
                proj_relu(2, 0, s2,
                          lambda hf: v_sb[:, 8 * s2 + 4 * hf:
                                          8 * s2 + 4 * hf + 4, 1, :],
                          bqkv_sb[:, 2:3])
                vw_mms(s2)

            exp_map = {sb: [None] * (MCHUNKS // 2) for sb in range(NSB)}

            def dots_slab(sbs, s2):
                """dps+exp for m-chunks of slab s2, for each query superblock.
                Every 4th tile (s2>0) bypasses the ACT engine: DVE applies the
                Schraudolph affine into int32, Pool bitcasts/converts to fp8."""
                for sb in sbs:
                    nsl = slice(sb * SB, (sb + 1) * SB)
                    for jl in range(4):
                        jj = s2 * 4 + jl
                        dps = ps.tile([128, 1024], f32, tag="ps", name="dps")
                        for u in (0, 1):
                            j = jj * 2 + u
                            nc.tensor.matmul(dps[:, u * 512:(u + 1) * 512],
                                             k2_sb[:, j * 128:(j + 1) * 128],
                                             q_sb[:, nsl], start=True, stop=True)
                        et = ep.tile([128, 1024], fp8, tag="exp", name="et")
                        if jl == 3 and sb >= 2 and (sb, s2) != (3, 3):
                            yt = yp.tile([128, 1024], i32, tag="y", name="yt")
                            nc.vector.tensor_scalar(yt[:], dps[:], EXPA, EXPB,
                                                    A.mult, A.add)
                            nc.gpsimd.tensor_copy(et[:], yt[:].bitcast(f32))
                        else:
                            nc.scalar.activation(et[:], dps[:], AF.Exp,
                                                 bias=ebias[:],
                                                 scale=DOTS_DESCALE)
                        exp_map[sb][jj] = et

            fps_map = {}    # (sb, t) -> (fps tile, n pairs accumulated)

            def pv_mms(sb, trange, jjs, fin):
                tiles = exp_map[sb]
                for t in trange:
                    fps, done = fps_map.get((sb, t), (None, 0))
                    if fps is None:
                        fps = sm.tile([128, 258], f32, tag="small", name="fps")
                    for jj in jjs:
                        etp = tiles[jj][:].rearrange("p (u n) -> p u n", u=2)
                        nc.tensor.matmul(fps[:], etp[:, :, t * 128:(t + 1) * 128],
                                         vw_sb[:, 2 * jj:2 * jj + 2, 0:258],
                                         start=(jj == 0),
                                         stop=(fin and jj == jjs[-1]),
                                         perf_mode=DR, skip_group_check=True)
                    fps_map[(sb, t)] = (fps, done + len(jjs))

            def pv_out(sb, trange=None):
                if trange is None:
                    trange = range(SB // 128)
                for t in trange:
                    nt = sb * 4 + t
                    _, done = fps_map.get((sb, t), (None, 0))
                    pv_mms(sb, (t,), range(done, 16), True)
                    fps, _ = fps_map.pop((sb, t))
                    rn = zp.tile([128, 1], f32, tag="rn")
                    nc.vector.reciprocal(rn[:], fps[:, 256:257])
                    t2 = zp.tile([128, OUT], f32, tag="t2")
                    nc.vector.scalar_tensor_tensor(t2[:], fps[:, 0:256], rn[:],
                                                   h_bc[:], A.mult, A.subtract)
                    z = zp.tile([128, OUT], f32, tag="z")
                    zeng = nc.vector if sb == 3 else nc.gpsimd
                    zeng.tensor_scalar(z[:], t2[:], -OUT_DESCALE, 0.0,
                                       A.mult, A.max)
                    nc.sync.dma_start(out[nt * 128:(nt + 1) * 128, :], z[:])

            # ---- phase 0: projections + VW, software-pipelined with dots ----
            # dots for slab s2-1 run while slab s2 is projected, so the dots
            # never wait on the current iteration's k-relu. The slab-0 k/q
            # relus run on the ACT engine, which is idle during the ramp.
            for s2 in range(4):
                proj_relu(1, 2, s2,
                          lambda hf: k2_sb[:, s2 * 1024 + hf * 512:
                                           s2 * 1024 + (hf + 1) * 512],
                          bqkv_sb[:, 1:2], act=(s2 == 0))
                if s2 == 0:
                    proj_relu(0, 0, 0,
                              lambda hf: q_sb[:, hf * 512:(hf + 1) * 512],
                              bqkv_sb[:, 0:1], act=True)
                else:
                    dots_slab((0,), s2 - 1)
                proj_relu(2, 2, s2,
                          lambda hf: v_sb[:, 8 * s2 + 4 * hf:
                                          8 * s2 + 4 * hf + 4, 0, :],
                          bqkv_sb[:, 2:3])
                if s2 > 0:
                    dots_slab((1,), s2 - 1)
                v1_block(s2, with_q=(s2 == 1))
            dots_slab((0, 1), 3)

            # ---- sumv totals, h ----
            sv2f = zp.tile([128, 1], f32, tag="svf")
            sv1f = zp.tile([128, 1], f32, tag="svf")
            nc.vector.tensor_reduce(sv2f[:], v_sb[:, :, 0, :], XY, A.add)
            nc.vector.tensor_reduce(sv1f[:], v_sb[:, :, 1, :], XY, A.add)
            hps = sm.tile([128, 258], f32, tag="small", name="hps")
            nc.tensor.matmul(hps[0:1, 0:256], sv2f[:], wph_sb[:, 0].bitcast(f32),
                             start=True, stop=False)
            nc.tensor.matmul(hps[0:1, 0:256], sv1f[:], wph_sb[:, 1].bitcast(f32),
                             start=False, stop=True)
            nc.vector.tensor_tensor(h_row[:], hps[0:1, 0:256], bp_sb[:], A.add)
            nc.gpsimd.partition_broadcast(h_bc[:], h_row[:])

            # ---- phase 1 steady state: PV tiles interleave with dots; sb3
            # pre-accumulates pairs 0..11 during PE slack so only 4 pair-
            # matmuls per tile remain after the final exp ----
            for s2 in range(4):
                dots_slab((2,), s2)
                pv_out(0, (s2,))
            for s2 in range(4):
                dots_slab((3,), s2)
                pv_out(1, (s2,))
                pv_out(2, (s2,))
            pv_mms(3, (0, 1), range(0, 12), False)
            pv_out(3)

    nc.compile()
    return nc


def _host_prep(inputs):
    E4 = ml_dtypes.float8_e4m3fn
    s_attn = np.float32(INTER ** -0.5)
    x1 = np.asarray(inputs["x1"], np.float32).reshape(B, C, N)
    x2 = np.asarray(inputs["x2"], np.float32).reshape(B, C, N)
    x1_8 = x1.astype(E4)
    x2_8 = x2.astype(E4)

    def eff(Wn, bn, sn, tn, extra=np.float32(1.0)):
        Wm = np.asarray(inputs[Wn], np.float32)
        bb = np.asarray(inputs[bn], np.float32)
        ss = np.asarray(inputs[sn], np.float32)
        tt = np.asarray(inputs[tn], np.float32)
        W_eff = (ss[:, None] * Wm) * extra
        b_eff = (ss * bb + tt) * extra
        return np.ascontiguousarray(W_eff.T), b_eff   # W_eff.T: [cin, cout]

    wqT, bqe = eff("Wq", "bq", "sq", "tq", s_attn * np.float32(AQ))
    wkT, bke = eff("Wk", "bk", "sk", "tk", np.float32(AK))
    wvT, bve = eff("Wv", "bv", "sv", "tv", np.float32(AV))
    wpT, bpe = eff("Wp", "bp", "sp", "tp", np.float32(AP_))

    # DoubleRow pair layout [cin128, proj, pair, cout]
    wqkv8 = np.stack([w.reshape(2, 128, 128).transpose(1, 0, 2)
                      for w in (wqT, wkT, wvT)], axis=1).astype(E4)
    wpT_pair = wpT.reshape(2, 128, OUT).transpose(1, 0, 2)
    wp8 = wpT_pair.astype(E4)
    wph = np.ascontiguousarray(wpT_pair, np.float32)   # true f32 Wp for h

    common = dict(
        wqkv=np.ascontiguousarray(wqkv8),
        wp8=np.ascontiguousarray(wp8),
        wph=np.ascontiguousarray(wph),
        bqkv=np.concatenate([bqe, bke, bve]).reshape(3 * INTER, 1),
        bp_row=(bpe * np.float32(AV)).reshape(1, OUT),
    )
    in_maps = []
    for c in range(NCORES):
        b, half = c // 2, c % 2
        # m-axis permutation: own query half first (identical for x1 and x2,
        # so all sum-over-m quantities are unchanged)
        perm = (np.r_[NQ:N, 0:NQ] if half else np.r_[0:N]).astype(np.intp)
        in_maps.append(dict(
            x12=np.ascontiguousarray(
                np.concatenate([x1_8[b][:, perm], x2_8[b][:, perm]], axis=0)),
            **common,
        ))
    return in_maps


def kernel(**inputs):
    from concourse.bass_utils import run_bass_kernel_spmd

    if "nc" not in _NC_CACHE:
        _NC_CACHE["nc"] = _build_nc()
    nc = _NC_CACHE["nc"]

    in_maps = _host_prep(inputs)
    res = run_bass_kernel_spmd(nc, in_maps, core_ids=list(range(NCORES)))

    full = np.empty((B, OUT, N), dtype=np.float32)
    for c in range(NCORES):
        b, half = c // 2, c % 2
        full[b][:, half * NQ:(half + 1) * NQ] = res.results[c]["out"].T
    return full.reshape(B, OUT, HH, WW)


if __name__ == "__main__":
    rng = np.random.default_rng(0)
    fake = {}
    fake["x1"] = rng.standard_normal((B, C, HH, WW), dtype=np.float32)
    fake["x2"] = rng.standard_normal((B, C, HH, WW), dtype=np.float32)
    for k, oc in (("q", INTER), ("k", INTER), ("v", INTER), ("p", OUT)):
        ic = C if k != "p" else 2 * INTER
        fake["W" + k] = rng.standard_normal((oc, ic), dtype=np.float32) * ic ** -0.5
        fake["b" + k] = np.zeros(oc, np.float32)
        fake["s" + k] = rng.uniform(0.5, 1.5, oc).astype(np.float32)
        fake["t" + k] = rng.standard_normal(oc, dtype=np.float32) * 0.1
    o = kernel(**fake)
    print("kernel ran, out shape", o.shape)
